# revision 4
# baseline (speedup 1.0000x reference)
"""Trainium2 Bass kernel for nn_Model_39676907882504.

Math: qk = (q @ k^T)/8 has shape [1,2048,1,1]; after the transposes it is
[2048,1,1,1], and softmax over the trailing size-1 axis is exactly 1.0
regardless of qk (exp(x-max)/sum == 1/1 bit-exactly). The final matmul
[S,Q,B,Q] @ [B,S,Q,D] with attn_weight == 1 therefore reduces to
broadcasting `value` across a new leading dim:

    output[i, j, 0, :] = value[0, j, 0, :]   for all i in [0, 2048)

i.e. a 512KB -> 1GiB broadcast copy.  Pure memory-regime kernel.

Sharding (per the hint): leading output dim (2048 rows) split across the
8 cores, 256 rows/core (= 4096 sub-rows of 32KB; sub-row d holds value
chunk d%16); value replicated.

Per-core plan (derived from trace analysis of earlier variants):

  * Every dma_start costs SDMA engine 15 a ~2.7us completion stall, so
    use as few instructions as possible: one load + TWO giant stores
    (one per HW-DGE queue), each 64MiB = 2048 descriptors of 32KB.
  * The SBUF tile is [64, 8192] at partitions 32..95: that partition
    range covers all 16 SBUF AXI ports with exactly 4 partitions each,
    and the contiguous 16-way descriptor split then gives every SDMA
    engine its own port (no contention) while the load is only 2MiB
    (4 descriptors/engine).  Partition 32+q holds chunk q%16 (4 copies
    of value).
  * Stores re-read the tile via a stride-0 middle dim [64, 32, 8192];
    the DRAM side is out.rearrange("(c q) e -> q c e", q=64) so
    descriptor (q, c) lands at sub-row q + 64*c, whose required content
    is chunk (q + 64*c)%16 = q%16.  The two instructions split the c
    axis.
  * The sync-queue store issues with no wait after the load: per-engine
    FIFO descriptor order guarantees each engine's 4 load descriptors
    complete ~3.6us before its first store descriptor re-reads that
    partition.  The scalar queue gates on the load semaphore.

Engine budget: 4 load + 256 store descriptors x ~1.216us = ~316us plus
~13us fixed NEFF entry/exit.
"""

import sys

for _p in ("/opt/trn_rl_repo",):
    if _p not in sys.path:
        sys.path.insert(0, _p)

import numpy as np

import concourse.bass as bass
import concourse.mybir as mybir
from concourse.bass_utils import run_bass_kernel_spmd

S = 2048
D = 64
N_CORES = 8
ROWS_PER_CORE = S // N_CORES          # 256 output rows/core, 512KB each
F = 8192                              # f32 per 32KB chunk; value = 16 chunks
SUBROWS = ROWS_PER_CORE * 16          # 4096 32KB sub-rows per shard
NREP = SUBROWS // 64                  # 64 broadcast reps of the 64-part tile
PBASE = 32                            # tile partitions 32..95 hit all 16 ports

TRACE = False          # test.py flips this to profile
TRACE_KWARGS = {}
LAST_RESULT = None     # BassKernelResults of the last run (for test.py)


def build_program():
    nc = bass.Bass()
    val = nc.declare_dram_parameter("value", [64, F], mybir.dt.float32,
                                    isOutput=False)
    out = nc.declare_dram_parameter("out", [SUBROWS, F], mybir.dt.float32,
                                    isOutput=True)

    vtile = nc.alloc_sbuf_tensor("vtile", [128, F], mybir.dt.float32)
    vslice = vtile[PBASE:PBASE + 64, :]

    # [q, c, e]: sub-row q + 64*c <- tile partition 32+q (chunk q%16).
    out_qce = out[:, :].rearrange("(c q) e -> q c e", q=64)
    half = NREP // 2

    def in_bcast(reps):
        return vslice.unsqueeze(1).broadcast_to((64, reps, F))

    with nc.Block() as block, \
         nc.semaphore("lsem") as lsem, \
         nc.semaphore("s1") as s1, \
         nc.semaphore("s2") as s2:

        @block.sync
        def _(sync):
            sync.dma_start(out=vslice, in_=val[:, :]).then_inc(lsem, 16)
            sync.dma_start(out=out_qce[:, 0:half, :],
                           in_=in_bcast(half)).then_inc(s1, 16)
            sync.wait_ge(s1, 16)

        @block.scalar
        def _(scalar):
            scalar.wait_ge(lsem, 16)
            scalar.dma_start(out=out_qce[:, half:NREP, :],
                             in_=in_bcast(NREP - half)).then_inc(s2, 16)
            scalar.wait_ge(s2, 16)

    return nc


def kernel(query=None, key=None, value=None, attn_mask=None, **_ignored):
    global LAST_RESULT
    value = np.ascontiguousarray(np.asarray(value, dtype=np.float32))
    vflat = value.reshape(16, F)                      # 16 chunks of 32KB
    vexp = np.ascontiguousarray(np.tile(vflat, (4, 1)))   # [64, F]

    nc = build_program()
    core_ids = list(range(N_CORES))
    in_maps = [{"value": vexp} for _ in core_ids]
    res = run_bass_kernel_spmd(nc, in_maps, core_ids, trace=TRACE,
                               **TRACE_KWARGS)
    LAST_RESULT = res

    # Core i supplies output rows [i*256, (i+1)*256).
    shards = [res.results[i]["out"].reshape(ROWS_PER_CORE, S, 1, D)
              for i in range(N_CORES)]
    return np.concatenate(shards, axis=0)
